# revision 19
# baseline (speedup 1.0000x reference)
"""Trainium2 kernel for nn_DPsoftminLoss.

Device (8 NeuronCores, batch-parallel): DPLoss[b,i,j] = sum_d |frame[b,i,d] -
DPmap[b,i,j,d]| -- this reads all 268MB of DPmap and is the memory-bound core.
Host: the T-sequential softmin/hardmin DP recurrence and backtracking, which
touch only the [B,T,J] cost map (1MB) and are inherently serial in T.
"""

import sys

sys.path.insert(0, "/opt/trn_rl_repo")

import numpy as np

B, T, J, D = 16, 128, 128, 256
NCORES = 8
BL = B // NCORES  # batch elements per core
JC = 8  # j-columns per streamed tile
NTILES = 9  # triangle-packed tiles per batch element (see _build_nc)
P_PENAL = np.float32(3.2)
TEMP2 = np.float32(0.01)

_nc_cache = {}


def _tile_layout():
    """Triangle packing: the DP only reads DPLoss[i, j] for j <= i, so only
    the lower triangle of each [T, J] plane is computed. Column-strip s
    (j in [8s, 8s+8)) needs rows i >= 8s. Strips s and 16-s together need
    (128-8s) + 8s = 128 rows, so they pack into one 128-partition tile:
      partitions [8s, 128) <- strip s     (row i = p)
      partitions [0,  8s)  <- strip 16-s  (row i = p + 128 - 8s)
    Tile list per batch element: [strip0 full, pairs (1,15)..(7,9), strip8].

    Returns per-tile (a_j0, a_p0, b_j0, b_rows): strip-A column start, A's
    first partition, strip-B column start (None if unpaired), B row count.
    """
    layout = [(0, 0, None, 0)]
    for s in range(1, 8):
        layout.append((8 * s, 8 * s, 128 - 8 * s, 8 * s))
    layout.append((64, 64, None, 0))
    return layout


def _build_nc():
    import concourse.bacc as bacc
    import concourse.bass as bass
    import concourse.mybir as mybir
    from concourse import tile

    # Bacc (not plain Bass): its compile() runs generate_event_semaphores,
    # which splits multi-semaphore waits to satisfy the 1-wait-per-
    # instruction hardware constraint.
    nc = bacc.Bacc()
    dpmap = nc.declare_dram_parameter("DPmap", [BL, T, J, D], mybir.dt.float32, isOutput=False)
    frame = nc.declare_dram_parameter("frame", [BL, T, D], mybir.dt.float32, isOutput=False)
    dpl_out = nc.declare_dram_parameter(
        "DPLraw", [BL, NTILES, T, JC], mybir.dt.float32, isOutput=True
    )

    def bcast_cols(ap2d):
        # [P, D] AP viewed as [P, JC, D] with a stride-0 (broadcast) middle dim
        return bass.AP(ap2d.tensor, ap2d.offset, [ap2d.ap[0], [0, JC], ap2d.ap[1]])

    with tile.TileContext(nc) as tc:
        with (
            tc.tile_pool(name="io", bufs=5) as io_pool,
            tc.tile_pool(name="misc", bufs=3) as misc_pool,
        ):
            for b in range(BL):
                ftile = misc_pool.tile([T, D], mybir.dt.float32, tag="ftile", bufs=2)
                nc.sync.dma_start(ftile[:], frame[b])
                for ti, (aj0, ap0, bj0, brows) in enumerate(_tile_layout()):
                    dpt = io_pool.tile([T, JC, D], mybir.dt.float32, tag="dpt")
                    nc.sync.dma_start(
                        dpt[ap0:, :, :], dpmap[b, ap0:, aj0 : aj0 + JC, :]
                    )
                    if bj0 is not None:
                        nc.sync.dma_start(
                            dpt[:brows, :, :], dpmap[b, bj0:, bj0 : bj0 + JC, :]
                        )
                        # frame rows matching the packed partition->row map
                        ftx = misc_pool.tile([T, D], mybir.dt.float32, tag="ftx")
                        nc.sync.dma_start(ftx[ap0:], frame[b, ap0:])
                        nc.sync.dma_start(ftx[:brows], frame[b, bj0:])
                        fsrc = ftx
                    else:
                        fsrc = ftile  # identity row map (rows < ap0 unused)
                    nc.vector.tensor_sub(dpt[:], dpt[:], bcast_cols(fsrc[:]))
                    # |diff| + sum over d, split between DVE (grouped fused
                    # abs-reduce) and ScalarE (per-column Abs + accum_out)
                    scr = misc_pool.tile([T, JC], mybir.dt.float32, tag="scr", bufs=4)
                    m = 3 if ti % 2 == 0 else 4
                    nc.vector.tensor_reduce(
                        scr[:, :m],
                        dpt[:, :m, :],
                        axis=mybir.AxisListType.X,
                        op=mybir.AluOpType.add,
                        apply_absolute_value=True,
                    )
                    for jc in range(m, JC):
                        nc.scalar.activation(
                            dpt[:, jc, :],
                            dpt[:, jc, :],
                            mybir.ActivationFunctionType.Abs,
                            accum_out=scr[:, jc : jc + 1],
                        )
                    nc.sync.dma_start(dpl_out[b, ti], scr[:])
    nc.finalize()  # Bacc: runs the pass pipeline incl. multi-wait splitting
    return nc


def _install_trace_shim():
    """Wire the NTFF profile hook that this container's boot left unregistered.

    Test-harness only (trace=True); the plain kernel path never calls this.
    """
    import sys as _sys
    import types

    try:
        import antenv.axon_hooks  # noqa: F401

        return
    except ImportError:
        pass
    if "/root/.axon_site" not in _sys.path:
        _sys.path.insert(0, "/root/.axon_site")
    from trn_agent_boot.trn_boot import _ntff_profile_via_ctypes

    hook = _ntff_profile_via_ctypes("/opt/axon/libaxon_pjrt.so")
    mod = types.ModuleType("antenv.axon_hooks")
    mod.get_axon_ntff_profile_hook = lambda: hook
    mod.set_axon_ntff_profile_hook = lambda h: None
    _sys.modules["antenv.axon_hooks"] = mod

    import concourse.bass_utils as bu

    bu.upload_artifacts = lambda tmpdir: "(local)"


def _run_device(DPmap, frame, trace=False):
    from concourse.bass_utils import run_bass_kernel_spmd

    if trace:
        try:
            _install_trace_shim()
        except Exception as e:  # profiling is best-effort in this container
            print(f"trace shim failed ({e}); running without trace")
            trace = False

    if "nc" not in _nc_cache:
        _nc_cache["nc"] = _build_nc()
    nc = _nc_cache["nc"]
    in_maps = [
        {
            "DPmap": np.ascontiguousarray(DPmap[c * BL : (c + 1) * BL]),
            "frame": np.ascontiguousarray(frame[c * BL : (c + 1) * BL]),
        }
        for c in range(NCORES)
    ]
    res = run_bass_kernel_spmd(nc, in_maps, list(range(NCORES)), trace=trace)
    raw = np.concatenate([res.results[c]["DPLraw"] for c in range(NCORES)], axis=0)
    return _unpack_dpl(raw), res


def _unpack_dpl(raw):
    """[B, NTILES, T, JC] packed strips -> [B, T, J] with zero upper triangle."""
    Bn = raw.shape[0]
    dpl = np.zeros((Bn, T, J), np.float32)
    for ti, (aj0, ap0, bj0, brows) in enumerate(_tile_layout()):
        dpl[:, ap0:, aj0 : aj0 + JC] = raw[:, ti, ap0:, :]
        if bj0 is not None:
            dpl[:, bj0:, bj0 : bj0 + JC] = raw[:, ti, :brows, :]
    return dpl


def _host_dp(DPLoss, framelen):
    """Sequential DP + backtracking on the [B,T,J] cost map, numpy.

    Uses the column decomposition cost[i,j] = G[j] + C[i,j] (j<=i) where
    C = cumsum of DPLoss along i and G[j] = diag[j] - C[j,j].
    """
    Bn, Tn, Jn = DPLoss.shape
    DPLoss = DPLoss.astype(np.float32)
    C = np.cumsum(DPLoss, axis=1)  # [B,T,J]
    ar = np.arange(Tn)
    diagC = C[:, ar, ar]  # C[b,j,j]
    dpl_diag = DPLoss[:, ar, ar]

    G = np.zeros((Bn, Tn), np.float32)
    G2 = np.zeros((Bn, Tn), np.float32)
    diag = np.zeros((Bn, Tn), np.float32)
    diag2 = np.zeros((Bn, Tn), np.float32)
    diag[:, 0] = DPLoss[:, 0, 0]
    diag2[:, 0] = DPLoss[:, 0, 0]
    G[:, 0] = diag[:, 0] - diagC[:, 0]
    G2[:, 0] = diag2[:, 0] - diagC[:, 0]
    for i in range(1, Tn):
        v = G[:, :i] + C[:, i - 1, :i]  # costMap[b, i-1, j] for j<i
        m = v.min(axis=1)
        lse = np.exp(-(v - m[:, None]) / TEMP2).sum(axis=1, dtype=np.float32)
        diag[:, i] = np.float32(-TEMP2) * np.log(lse) + P_PENAL + dpl_diag[:, i] + m
        G[:, i] = diag[:, i] - diagC[:, i]
        v2 = G2[:, :i] + C[:, i - 1, :i]
        diag2[:, i] = v2.min(axis=1) + P_PENAL + dpl_diag[:, i]
        G2[:, i] = diag2[:, i] - diagC[:, i]

    bidx = np.arange(Bn)
    fl1 = framelen.astype(np.int64) - 1
    maskF = np.arange(Jn)[None, :] < framelen[:, None]

    k = G + C[bidx, fl1]  # costMap[b, fl1, j] for j<=fl1
    km = np.where(maskF, k, np.inf)
    mk = km.min(axis=1)
    expo = np.where(maskF, -(k - mk[:, None]) / TEMP2, np.float32(-np.inf))
    s = np.exp(expo).sum(axis=1, dtype=np.float32)
    ans = (np.float32(-TEMP2) * np.log(s) + mk).sum(dtype=np.float32)

    # backtracking on the hard-min map
    k2 = G2 + C[bidx, fl1]
    lastindexEmb = np.where(maskF, k2, np.inf).argmin(axis=1)
    cost2 = G2[:, None, :] + C  # [B,T,J]; valid where j<=r
    rowsm = np.where(np.arange(Jn)[None, None, :] <= ar[None, :, None], cost2, np.inf)
    argp = rowsm.argmin(axis=2)  # [B,T] first occurrence

    save_init_last = np.zeros((Bn, Jn), np.float32)
    save_init_last[bidx[fl1 == Tn - 1], lastindexEmb[fl1 == Tn - 1]] = 1.0
    rows = np.zeros((Bn, Tn, Jn), np.float32)
    rows[:, Tn - 1] = save_init_last
    row_next = save_init_last
    for r in range(Tn - 2, -1, -1):
        scal = row_next[:, r + 1]  # savepos[:, r+1, r+1]
        orig_row = np.zeros((Bn, Jn), np.float32)
        sel = fl1 == r
        orig_row[bidx[sel], lastindexEmb[sel]] = 1.0
        onehot = np.zeros((Bn, Jn), np.float32)
        onehot[bidx, argp[:, r]] = 1.0
        row_next = orig_row + scal[:, None] * onehot + row_next
        rows[:, r] = row_next

    d = np.arange(min(Tn, Jn))
    switchpos = (rows[:, d, d] == 1.0).astype(np.int32)
    return np.float32(ans), switchpos


def kernel(DPmap, frame, framelen, _trace=False, _return_res=False):
    DPmap = np.asarray(DPmap, dtype=np.float32)
    frame = np.asarray(frame, dtype=np.float32)
    framelen = np.asarray(framelen, dtype=np.int32)
    dpl, res = _run_device(DPmap, frame, trace=_trace)
    ans, switchpos = _host_dp(dpl, framelen)
    if _return_res:
        return (ans, switchpos), res, dpl
    return ans, switchpos


# revision 22
# speedup vs baseline: 1.2149x; 1.2149x over previous
"""Trainium2 kernel for nn_DPsoftminLoss.

Device (8 NeuronCores, batch-parallel): DPLoss[b,i,j] = sum_d |frame[b,i,d] -
DPmap[b,i,j,d]| -- this reads all 268MB of DPmap and is the memory-bound core.
Host: the T-sequential softmin/hardmin DP recurrence and backtracking, which
touch only the [B,T,J] cost map (1MB) and are inherently serial in T.
"""

import sys

sys.path.insert(0, "/opt/trn_rl_repo")

import numpy as np

B, T, J, D = 16, 128, 128, 256
NCORES = 8
BL = B // NCORES  # batch elements per core
JC = 8  # j-columns per streamed tile
NTILES = 9  # triangle-packed tiles per batch element (see _build_nc)
P_PENAL = np.float32(3.2)
TEMP2 = np.float32(0.01)

_nc_cache = {}


def _tile_layout():
    """Triangle packing: the DP only reads DPLoss[i, j] for j <= i, so only
    the lower triangle of each [T, J] plane is computed. Column-strip s
    (j in [8s, 8s+8)) needs rows i >= 8s. Strips s and 16-s together need
    (128-8s) + 8s = 128 rows, so they pack into one 128-partition tile:
      partitions [8s, 128) <- strip s     (row i = p)
      partitions [0,  8s)  <- strip 16-s  (row i = p + 128 - 8s)
    Tile list per batch element: [strip0 full, pairs (1,15)..(7,9), strip8].

    Returns per-tile (a_j0, a_p0, b_j0, b_rows): strip-A column start, A's
    first partition, strip-B column start (None if unpaired), B row count.
    """
    layout = [(0, 0, None, 0)]
    for s in range(1, 8):
        layout.append((8 * s, 8 * s, 128 - 8 * s, 8 * s))
    layout.append((64, 64, None, 0))
    return layout


def _build_nc():
    import concourse.bacc as bacc
    import concourse.bass as bass
    import concourse.mybir as mybir
    from concourse import tile

    # Bacc (not plain Bass): its compile() runs generate_event_semaphores,
    # which splits multi-semaphore waits to satisfy the 1-wait-per-
    # instruction hardware constraint.
    nc = bacc.Bacc()
    dpmap = nc.declare_dram_parameter("DPmap", [BL, T, J, D], mybir.dt.float32, isOutput=False)
    frame = nc.declare_dram_parameter("frame", [BL, T, D], mybir.dt.float32, isOutput=False)
    dpl_out = nc.declare_dram_parameter(
        "DPLraw", [BL, NTILES, T, JC], mybir.dt.float32, isOutput=True
    )

    def bcast_cols(ap2d):
        # [P, D] AP viewed as [P, JC, D] with a stride-0 (broadcast) middle dim
        return bass.AP(ap2d.tensor, ap2d.offset, [ap2d.ap[0], [0, JC], ap2d.ap[1]])

    with tile.TileContext(nc) as tc:
        with (
            tc.tile_pool(name="io", bufs=8) as io_pool,
            tc.tile_pool(name="misc", bufs=3) as misc_pool,
        ):
            for b in range(BL):
                ftile = misc_pool.tile([T, D], mybir.dt.float32, tag="ftile", bufs=2)
                nc.sync.dma_start(ftile[:], frame[b])
                for ti, (aj0, ap0, bj0, brows) in enumerate(_tile_layout()):
                    dpt = io_pool.tile([T, JC, D], mybir.dt.float32, tag="dpt")
                    nc.sync.dma_start(
                        dpt[ap0:, :, :], dpmap[b, ap0:, aj0 : aj0 + JC, :]
                    )
                    if bj0 is not None:
                        nc.sync.dma_start(
                            dpt[:brows, :, :], dpmap[b, bj0:, bj0 : bj0 + JC, :]
                        )
                        # frame rows matching the packed partition->row map
                        ftx = misc_pool.tile([T, D], mybir.dt.float32, tag="ftx")
                        nc.sync.dma_start(ftx[ap0:], frame[b, ap0:])
                        nc.sync.dma_start(ftx[:brows], frame[b, bj0:])
                        fsrc = ftx
                    else:
                        fsrc = ftile  # identity row map (rows < ap0 unused)
                    nc.vector.tensor_sub(dpt[:], dpt[:], bcast_cols(fsrc[:]))
                    # |diff| + sum over d, split between DVE (grouped fused
                    # abs-reduce) and ScalarE (per-column Abs + accum_out)
                    scr = misc_pool.tile([T, JC], mybir.dt.float32, tag="scr", bufs=6)
                    m = 4 if ti % 2 == 0 else 5
                    nc.vector.tensor_reduce(
                        scr[:, :m],
                        dpt[:, :m, :],
                        axis=mybir.AxisListType.X,
                        op=mybir.AluOpType.add,
                        apply_absolute_value=True,
                    )
                    for jc in range(m, JC):
                        nc.scalar.activation(
                            dpt[:, jc, :],
                            dpt[:, jc, :],
                            mybir.ActivationFunctionType.Abs,
                            accum_out=scr[:, jc : jc + 1],
                        )
                    # result write-out from GpSimd (idle): keeps the Sync
                    # engine's stream pure input DMAs -- an SP-issued output
                    # DMA would block SP on the ACT accumulator semaphore
                    # and stall all downstream input-DMA issuance
                    nc.gpsimd.dma_start(dpl_out[b, ti], scr[:])
    nc.finalize()  # Bacc: runs the pass pipeline incl. multi-wait splitting
    return nc


def _install_trace_shim():
    """Wire the NTFF profile hook that this container's boot left unregistered.

    Test-harness only (trace=True); the plain kernel path never calls this.
    """
    import sys as _sys
    import types

    try:
        import antenv.axon_hooks  # noqa: F401

        return
    except ImportError:
        pass
    if "/root/.axon_site" not in _sys.path:
        _sys.path.insert(0, "/root/.axon_site")
    from trn_agent_boot.trn_boot import _ntff_profile_via_ctypes

    hook = _ntff_profile_via_ctypes("/opt/axon/libaxon_pjrt.so")
    mod = types.ModuleType("antenv.axon_hooks")
    mod.get_axon_ntff_profile_hook = lambda: hook
    mod.set_axon_ntff_profile_hook = lambda h: None
    _sys.modules["antenv.axon_hooks"] = mod

    import concourse.bass_utils as bu

    bu.upload_artifacts = lambda tmpdir: "(local)"


def _run_device(DPmap, frame, trace=False):
    from concourse.bass_utils import run_bass_kernel_spmd

    if trace:
        try:
            _install_trace_shim()
        except Exception as e:  # profiling is best-effort in this container
            print(f"trace shim failed ({e}); running without trace")
            trace = False

    if "nc" not in _nc_cache:
        _nc_cache["nc"] = _build_nc()
    nc = _nc_cache["nc"]
    in_maps = [
        {
            "DPmap": np.ascontiguousarray(DPmap[c * BL : (c + 1) * BL]),
            "frame": np.ascontiguousarray(frame[c * BL : (c + 1) * BL]),
        }
        for c in range(NCORES)
    ]
    res = run_bass_kernel_spmd(nc, in_maps, list(range(NCORES)), trace=trace)
    raw = np.concatenate([res.results[c]["DPLraw"] for c in range(NCORES)], axis=0)
    return _unpack_dpl(raw), res


def _unpack_dpl(raw):
    """[B, NTILES, T, JC] packed strips -> [B, T, J] with zero upper triangle."""
    Bn = raw.shape[0]
    dpl = np.zeros((Bn, T, J), np.float32)
    for ti, (aj0, ap0, bj0, brows) in enumerate(_tile_layout()):
        dpl[:, ap0:, aj0 : aj0 + JC] = raw[:, ti, ap0:, :]
        if bj0 is not None:
            dpl[:, bj0:, bj0 : bj0 + JC] = raw[:, ti, :brows, :]
    return dpl


def _host_dp(DPLoss, framelen):
    """Sequential DP + backtracking on the [B,T,J] cost map, numpy.

    Uses the column decomposition cost[i,j] = G[j] + C[i,j] (j<=i) where
    C = cumsum of DPLoss along i and G[j] = diag[j] - C[j,j].
    """
    Bn, Tn, Jn = DPLoss.shape
    DPLoss = DPLoss.astype(np.float32)
    C = np.cumsum(DPLoss, axis=1)  # [B,T,J]
    ar = np.arange(Tn)
    diagC = C[:, ar, ar]  # C[b,j,j]
    dpl_diag = DPLoss[:, ar, ar]

    G = np.zeros((Bn, Tn), np.float32)
    G2 = np.zeros((Bn, Tn), np.float32)
    diag = np.zeros((Bn, Tn), np.float32)
    diag2 = np.zeros((Bn, Tn), np.float32)
    diag[:, 0] = DPLoss[:, 0, 0]
    diag2[:, 0] = DPLoss[:, 0, 0]
    G[:, 0] = diag[:, 0] - diagC[:, 0]
    G2[:, 0] = diag2[:, 0] - diagC[:, 0]
    for i in range(1, Tn):
        v = G[:, :i] + C[:, i - 1, :i]  # costMap[b, i-1, j] for j<i
        m = v.min(axis=1)
        lse = np.exp(-(v - m[:, None]) / TEMP2).sum(axis=1, dtype=np.float32)
        diag[:, i] = np.float32(-TEMP2) * np.log(lse) + P_PENAL + dpl_diag[:, i] + m
        G[:, i] = diag[:, i] - diagC[:, i]
        v2 = G2[:, :i] + C[:, i - 1, :i]
        diag2[:, i] = v2.min(axis=1) + P_PENAL + dpl_diag[:, i]
        G2[:, i] = diag2[:, i] - diagC[:, i]

    bidx = np.arange(Bn)
    fl1 = framelen.astype(np.int64) - 1
    maskF = np.arange(Jn)[None, :] < framelen[:, None]

    k = G + C[bidx, fl1]  # costMap[b, fl1, j] for j<=fl1
    km = np.where(maskF, k, np.inf)
    mk = km.min(axis=1)
    expo = np.where(maskF, -(k - mk[:, None]) / TEMP2, np.float32(-np.inf))
    s = np.exp(expo).sum(axis=1, dtype=np.float32)
    ans = (np.float32(-TEMP2) * np.log(s) + mk).sum(dtype=np.float32)

    # backtracking on the hard-min map
    k2 = G2 + C[bidx, fl1]
    lastindexEmb = np.where(maskF, k2, np.inf).argmin(axis=1)
    cost2 = G2[:, None, :] + C  # [B,T,J]; valid where j<=r
    rowsm = np.where(np.arange(Jn)[None, None, :] <= ar[None, :, None], cost2, np.inf)
    argp = rowsm.argmin(axis=2)  # [B,T] first occurrence

    save_init_last = np.zeros((Bn, Jn), np.float32)
    save_init_last[bidx[fl1 == Tn - 1], lastindexEmb[fl1 == Tn - 1]] = 1.0
    rows = np.zeros((Bn, Tn, Jn), np.float32)
    rows[:, Tn - 1] = save_init_last
    row_next = save_init_last
    for r in range(Tn - 2, -1, -1):
        scal = row_next[:, r + 1]  # savepos[:, r+1, r+1]
        orig_row = np.zeros((Bn, Jn), np.float32)
        sel = fl1 == r
        orig_row[bidx[sel], lastindexEmb[sel]] = 1.0
        onehot = np.zeros((Bn, Jn), np.float32)
        onehot[bidx, argp[:, r]] = 1.0
        row_next = orig_row + scal[:, None] * onehot + row_next
        rows[:, r] = row_next

    d = np.arange(min(Tn, Jn))
    switchpos = (rows[:, d, d] == 1.0).astype(np.int32)
    return np.float32(ans), switchpos


def kernel(DPmap, frame, framelen, _trace=False, _return_res=False):
    DPmap = np.asarray(DPmap, dtype=np.float32)
    frame = np.asarray(frame, dtype=np.float32)
    framelen = np.asarray(framelen, dtype=np.int32)
    dpl, res = _run_device(DPmap, frame, trace=_trace)
    ans, switchpos = _host_dp(dpl, framelen)
    if _return_res:
        return (ans, switchpos), res, dpl
    return ans, switchpos
